# revision 5
# baseline (speedup 1.0000x reference)
"""Multi-head attention (B=2, S=2048, HID=1024, NH=16, DH=64) on 8 trn2 cores.

Sharding: tensor-parallel over (batch, head-group): core c handles batch c//4
and heads 4*(c%4)..4*(c%4)+3 (256 of the 1024 hidden dims). Each core computes
q/k/v projections for its heads, attention, and a partial output projection;
the host sums the 4 partials per batch and adds the output bias.

Layout strategy (all matmuls are float32r = fp32 storage, FP22 multiply,
fp32 accumulate — full PE speed):
  - Host pre-transposes Q/K/V ([HID, S] per batch) and weights so no on-device
    transposes are needed.
  - qT, kT are kept head-major [dh, S]; scores are computed transposed
    (sT[key, query] = kT.T @ qT) with two heads packed into the 128-wide PE
    contraction via row-group tile_position.
  - exp(sT) tiles feed ctxT = v_aug.T @ expT where v_aug has a ones column, so
    the softmax denominator Z accumulates in PSUM row 64 for free.
  - The reference's mask quirk (masked scores set to -1e-9, NOT -inf) makes a
    masked entry contribute exp(-1e-9) == 1.0f; softmax max-subtraction is
    skipped (scores are O(1), exp is safe) so masked entries are exactly 1.0.
    Future-key blocks are therefore never computed: their contribution is a
    rank-1 update (suffix-sums of v_aug) added straight into the ctx PSUM.
"""

import numpy as np

B, S, HID, NH, DH = 2, 2048, 1024, 16, 64
NCORES = 8
GROUPS = 4            # head groups (cores per batch)
HPC = NH // GROUPS    # 4 heads per core
HDS = HPC * DH        # 256 hidden dims per core
QW = 512              # query-chunk width (one fp32 PSUM bank)
NCQ = S // QW         # 4 query chunks
NKT = S // 128        # 16 key tiles

_progs = {}


def _split_sync_waits(nc, max_waits: int = 1) -> int:
    """neuronxcc walrus codegen rejects instructions with more than one sync
    wait ("Too many sync wait commands"). Move excess waits onto preceding
    same-engine NoOps."""
    import concourse.mybir as mybir

    n_split = 0
    for fn in nc.m.functions:
        for bb in fn.blocks:
            out = []
            for ins in bb.instructions:
                si = ins.sync_info
                if si is not None and si.on_wait and len(si.on_wait) > max_waits:
                    waits = list(si.on_wait)
                    extra, keep = waits[:-max_waits], waits[-max_waits:]
                    for i in range(0, len(extra), max_waits):
                        chunk = extra[i:i + max_waits]
                        nop = mybir.InstNoOp(
                            name=nc.get_next_instruction_name(),
                            engine=ins.engine,
                            ins=[],
                            outs=[],
                            sync_info=mybir.SyncInfo(on_wait=chunk, on_update=[]),
                            bass_nofuse=True,
                            text_hint="split_sync_waits",
                        )
                        out.append(nop)
                        n_split += 1
                    si.on_wait = keep
                out.append(ins)
            bb.instructions[:] = out
    return n_split


def _build_program(causal: bool):
    import concourse.bass as bass
    import concourse.tile as tile
    from concourse import mybir

    f32 = mybir.dt.float32
    f32r = mybir.dt.float32r
    Ident = mybir.ActivationFunctionType.Identity
    Copy = mybir.ActivationFunctionType.Copy
    Exp = mybir.ActivationFunctionType.Exp

    nc = bass.Bass()
    qt_d = nc.dram_tensor("qt", [HID, S], f32r, kind="ExternalInput")
    kt_d = nc.dram_tensor("kt", [HID, S], f32r, kind="ExternalInput")
    vt_d = nc.dram_tensor("vt", [HID, S], f32r, kind="ExternalInput")
    wq_d = nc.dram_tensor("wq", [HID, HDS], f32r, kind="ExternalInput")
    wk_d = nc.dram_tensor("wk", [HID, HDS], f32r, kind="ExternalInput")
    wv_d = nc.dram_tensor("wv", [HID, HDS], f32r, kind="ExternalInput")
    wo_d = nc.dram_tensor("wo", [HDS, HID], f32r, kind="ExternalInput")
    bq_d = nc.dram_tensor("bq", [128, 2], f32, kind="ExternalInput")
    bk_d = nc.dram_tensor("bk", [128, 2], f32, kind="ExternalInput")
    bv_d = nc.dram_tensor("bv", [1, HDS], f32r, kind="ExternalInput")
    on_d = nc.dram_tensor("on", [128, QW], f32r, kind="ExternalInput")
    mk_d = nc.dram_tensor("mk", [128, 128], f32, kind="ExternalInput")
    out_d = nc.dram_tensor("out", [S, HID], f32, kind="ExternalOutput")

    qt_r = qt_d.rearrange("(ko p) s -> p ko s", p=128)
    kt_r = kt_d.rearrange("(ko p) s -> p ko s", p=128)
    vt_r = vt_d.rearrange("(ko p) s -> p ko s", p=128)

    with tile.TileContext(nc) as tc:
        with tc.tile_pool(name="persist", bufs=1) as persist:
            qT = persist.tile([128, 2, S], f32r)       # [dh(2 heads), m, s]
            kT = persist.tile([128, 2, S], f32r)
            vA = persist.tile([128, NKT, HPC, DH + 1], f32r)  # v_aug per s-tile
            cT = persist.tile([128, 2, S], f32r)       # ctxT (divided by Z)
            ones = persist.tile([128, QW], f32r)
            maskt = persist.tile([128, 128], f32)
            bq_sb = persist.tile([128, 2], f32)
            bk_sb = persist.tile([128, 2], f32)
            bv_sb = persist.tile([1, HDS], f32r)
            vs_sb = persist.tile([1, 3, HPC * (DH + 1)], f32r)
            wo_sb = persist.tile([128, 2, HID], f32r)

            nc.sync.dma_start(ones[:], on_d[:])
            nc.sync.dma_start(maskt[:], mk_d[:])
            nc.sync.dma_start(bq_sb[:], bq_d[:])
            nc.sync.dma_start(bk_sb[:], bk_d[:])
            nc.sync.dma_start(bv_sb[:], bv_d[:])
            nc.sync.dma_start(wo_sb[:], wo_d.rearrange("(ko p) o -> p ko o", p=128))

            # ---------- Phase A: projections ----------
            with (
                tc.tile_pool(name="wpool", bufs=1) as wpool,
                tc.tile_pool(name="app", bufs=4, space="PSUM") as app,
                tc.tile_pool(name="vpp", bufs=2, space="PSUM") as vpp,
                tc.tile_pool(name="arhs", bufs=2) as arhs,
                tc.tile_pool(name="vsl", bufs=3) as vsl,
            ):
                wq_sb = wpool.tile([128, 8, HDS], f32r, tag="wq")
                wk_sb = wpool.tile([128, 8, HDS], f32r, tag="wk")
                wv_sb = wpool.tile([128, 8, HDS], f32r, tag="wv")
                nc.sync.dma_start(wq_sb[:], wq_d.rearrange("(ko p) m -> p ko m", p=128))
                nc.sync.dma_start(wk_sb[:], wk_d.rearrange("(ko p) m -> p ko m", p=128))
                nc.sync.dma_start(wv_sb[:], wv_d.rearrange("(ko p) m -> p ko m", p=128))

                for src_r, w_sb, b_sb, dstT in (
                    (qt_r, wq_sb, bq_sb, qT),
                    (kt_r, wk_sb, bk_sb, kT),
                ):
                    for ns in range(4):
                        rh = arhs.tile([128, 8, QW], f32r, tag="projrhs")
                        nc.sync.dma_start(rh[:], src_r[:, :, ns * QW:(ns + 1) * QW])
                        for m in range(2):
                            ps = app.tile([128, QW], f32, tag="projps")
                            for ko in range(8):
                                nc.tensor.matmul(
                                    ps[:],
                                    w_sb[:, ko, m * 128:(m + 1) * 128],
                                    rh[:, ko, :],
                                    start=(ko == 0),
                                    stop=(ko == 7),
                                )
                            nc.scalar.activation(
                                dstT[:, m, ns * QW:(ns + 1) * QW], ps[:],
                                Ident, bias=b_sb[:, m:m + 1],
                            )

                # ones column of v_aug (written once, before per-tile evacs)
                nc.scalar.activation(
                    vA[:, :, :, DH:DH + 1],
                    ones[:, 0:NKT * HPC].rearrange("p (a b o) -> p a b o", a=NKT, b=HPC),
                    Copy,
                )
                for st in range(NKT):
                    vslab = vsl.tile([128, 8, 128], f32r, tag="vslab")
                    nc.sync.dma_start(vslab[:], vt_r[:, :, st * 128:(st + 1) * 128])
                    ps = vpp.tile([128, HDS], f32, tag="vps")
                    for ko in range(8):
                        nc.tensor.matmul(
                            ps[:], vslab[:, ko, :], wv_sb[:, ko, :],
                            start=(ko == 0), stop=False,
                        )
                    nc.tensor.matmul(
                        ps[:], ones[0:1, 0:128], bv_sb[0:1, :],
                        start=False, stop=True,
                    )
                    nc.scalar.activation(
                        vA[:, st, :, 0:DH],
                        ps.rearrange("p (h d) -> p h d", h=HPC),
                        Ident,
                    )

                if causal:
                    # suffix sums of v_aug column-totals: vs_sb[0, c-1, :] =
                    # sum_{st >= 4c} colsum(v_aug[st])  (includes key counts)
                    for c in (1, 2, 3):
                        vps = vpp.tile([1, HPC * (DH + 1)], f32, tag="vsps")
                        for st in range(4 * c, NKT):
                            nc.tensor.matmul(
                                vps[:],
                                ones[:, 0:1],
                                vA[:, st, :, :].rearrange("p a b -> p (a b)"),
                                start=(st == 4 * c),
                                stop=(st == NKT - 1),
                            )
                        nc.vector.tensor_copy(vs_sb[0:1, c - 1, :], vps[:])

            # ---------- Phase B: attention ----------
            with (
                tc.tile_pool(name="sps", bufs=4, space="PSUM") as sps,
                tc.tile_pool(name="cps", bufs=2, space="PSUM") as cps,
                tc.tile_pool(name="zps", bufs=2, space="PSUM") as zps,
                tc.tile_pool(name="esb", bufs=4) as esb,
                tc.tile_pool(name="fin", bufs=2) as fin,
            ):
                for hp in range(2):
                    for cq in range(NCQ):
                        n_kt = 4 * (cq + 1) if causal else NKT
                        last_on_loop = (not causal) or cq == 3
                        ctx = [
                            cps.tile([DH + 1, QW], f32, tag="ctx", name=f"ctx{i}")
                            for i in range(2)
                        ]
                        for kt_i in range(n_kt):
                            es = []
                            for hl in range(2):
                                lo = 64 * hl
                                s_ps = sps.tile([128, QW], f32, tag="s")
                                nc.tensor.matmul(
                                    s_ps[:],
                                    kT[lo:lo + 64, hp, kt_i * 128:(kt_i + 1) * 128],
                                    qT[lo:lo + 64, hp, cq * QW:(cq + 1) * QW],
                                    start=True, stop=True,
                                    tile_position=(lo, 0),
                                )
                                r = kt_i - 4 * cq
                                if causal and r >= 0:
                                    nc.vector.tensor_mul(
                                        s_ps[:, r * 128:(r + 1) * 128],
                                        s_ps[:, r * 128:(r + 1) * 128],
                                        maskt[:],
                                    )
                                e = esb.tile([128, QW], f32r, tag="e")
                                nc.scalar.activation(e[:], s_ps[:], Exp, scale=0.125)
                                if causal and r >= 1:
                                    nc.vector.tensor_copy(
                                        e[:, 0:r * 128], ones[:, 0:r * 128]
                                    )
                                es.append(e)
                            for hl in range(2):
                                h = 2 * hp + hl
                                nc.tensor.matmul(
                                    ctx[hl][:],
                                    vA[:, kt_i, h, :],
                                    es[hl][:],
                                    start=(kt_i == 0),
                                    stop=(kt_i == n_kt - 1 and last_on_loop),
                                )
                        if causal and cq < 3:
                            for hl in range(2):
                                h = 2 * hp + hl
                                nc.tensor.matmul(
                                    ctx[hl][:],
                                    vs_sb[0:1, cq, (DH + 1) * h:(DH + 1) * (h + 1)],
                                    ones[0:1, 0:QW],
                                    start=False, stop=True,
                                )
                        for hl in range(2):
                            zr = fin.tile([DH + 1, QW], f32r, tag="zr")
                            with nc.allow_low_precision(reason="f32r rhs"):
                                nc.vector.reciprocal(
                                    zr[DH:DH + 1, :], ctx[hl][DH:DH + 1, :]
                                )
                            zb = zps.tile([DH, QW], f32, tag="zb")
                            nc.tensor.matmul(
                                zb[:], ones[64:65, 0:DH], zr[DH:DH + 1, :],
                                start=True, stop=True,
                            )
                            zc = fin.tile([DH, QW], f32, tag="zc")
                            nc.vector.tensor_copy(zc[:], zb[:])
                            nc.vector.tensor_mul(
                                cT[64 * hl:64 * (hl + 1), hp, cq * QW:(cq + 1) * QW],
                                ctx[hl][0:DH, :],
                                zc[:],
                            )

            # ---------- Phase C: output projection (partial) ----------
            with (
                tc.tile_pool(name="ops", bufs=4, space="PSUM") as ops_p,
                tc.tile_pool(name="osb", bufs=3) as osb,
            ):
                for q_i in range(NKT):
                    ost = osb.tile([128, 2, QW], f32, tag="ost")
                    for no in range(2):
                        ps = ops_p.tile([128, QW], f32, tag="ops")
                        for ko in range(2):
                            nc.tensor.matmul(
                                ps[:],
                                cT[:, ko, q_i * 128:(q_i + 1) * 128],
                                wo_sb[:, ko, no * QW:(no + 1) * QW],
                                start=(ko == 0), stop=(ko == 1),
                            )
                        nc.vector.tensor_copy(ost[:, no, :], ps[:])
                    nc.sync.dma_start(
                        out_d[q_i * 128:(q_i + 1) * 128, :],
                        ost.rearrange("p a b -> p (a b)"),
                    )

    _split_sync_waits(nc)
    return nc


def _get_program(causal: bool):
    if causal not in _progs:
        _progs[causal] = _build_program(causal)
    return _progs[causal]


def _numpy_fallback(Q, K, V, pad_mask, attn_mask, Wq, bq, Wk, bk, Wv, bv, Wo, bo):
    NEG = np.float32(-1e-09)

    def split_heads(x):
        return x.reshape(B, S, NH, DH).transpose(0, 2, 1, 3)

    q = split_heads(Q @ Wq.T + bq)
    k = split_heads(K @ Wk.T + bk)
    v = split_heads(V @ Wv.T + bv)
    scores = np.einsum("bhqd,bhkd->bhqk", q, k) / np.sqrt(DH)
    mask = pad_mask[:, :, None] * pad_mask[:, None, :] * attn_mask
    scores = np.where(mask[:, None, :, :] != 0, scores, NEG)
    scores = scores - scores.max(axis=-1, keepdims=True)
    e = np.exp(scores)
    attn = e / e.sum(axis=-1, keepdims=True)
    ctx = np.einsum("bhqk,bhkd->bhqd", attn, v)
    ctx = ctx.transpose(0, 2, 1, 3).reshape(B, S, HID)
    return (ctx @ Wo.T + bo).astype(np.float32)


def kernel(Q, K, V, pad_mask, attn_mask, Wq, bq, Wk, bk, Wv, bv, Wo, bo):
    Q = np.asarray(Q, np.float32)
    K = np.asarray(K, np.float32)
    V = np.asarray(V, np.float32)
    pad_mask = np.asarray(pad_mask, np.float32)
    attn_mask = np.asarray(attn_mask, np.float32)
    Wq = np.asarray(Wq, np.float32)
    bq = np.asarray(bq, np.float32)
    Wk = np.asarray(Wk, np.float32)
    bk = np.asarray(bk, np.float32)
    Wv = np.asarray(Wv, np.float32)
    bv = np.asarray(bv, np.float32)
    Wo = np.asarray(Wo, np.float32)
    bo = np.asarray(bo, np.float32)

    tril = np.tril(np.ones((S, S), np.float32))
    if not np.all(pad_mask == 1.0):
        return _numpy_fallback(Q, K, V, pad_mask, attn_mask,
                               Wq, bq, Wk, bk, Wv, bv, Wo, bo)
    if np.array_equal(attn_mask, tril):
        causal = True
    elif np.all(attn_mask != 0.0):
        causal = False
    else:
        return _numpy_fallback(Q, K, V, pad_mask, attn_mask,
                               Wq, bq, Wk, bk, Wv, bv, Wo, bo)

    from concourse.bass_utils import run_bass_kernel_spmd

    nc = _get_program(causal)

    ones = np.ones((128, QW), np.float32)
    maskt = np.triu(np.ones((128, 128), np.float32))  # [key, query]: key<=query
    WqT = np.ascontiguousarray(Wq.T)
    WkT = np.ascontiguousarray(Wk.T)
    WvT = np.ascontiguousarray(Wv.T)
    WoT = np.ascontiguousarray(Wo.T)
    QT = [np.ascontiguousarray(Q[b].T) for b in range(B)]
    KT = [np.ascontiguousarray(K[b].T) for b in range(B)]
    VT = [np.ascontiguousarray(V[b].T) for b in range(B)]

    in_maps = []
    for c in range(NCORES):
        b, g = divmod(c, GROUPS)
        hs = slice(g * HDS, (g + 1) * HDS)
        in_maps.append({
            "qt": QT[b], "kt": KT[b], "vt": VT[b],
            "wq": np.ascontiguousarray(WqT[:, hs]),
            "wk": np.ascontiguousarray(WkT[:, hs]),
            "wv": np.ascontiguousarray(WvT[:, hs]),
            "wo": np.ascontiguousarray(WoT[hs, :]),
            "bq": np.ascontiguousarray(bq[hs].reshape(2, 128).T),
            "bk": np.ascontiguousarray(bk[hs].reshape(2, 128).T),
            "bv": np.ascontiguousarray(bv[hs].reshape(1, HDS)),
            "on": ones, "mk": maskt,
        })

    global _trace_in_maps
    _trace_in_maps = in_maps

    res = run_bass_kernel_spmd(nc, in_maps, core_ids=list(range(NCORES)))
    out = np.empty((B, S, HID), np.float32)
    for b in range(B):
        acc = res.results[GROUPS * b]["out"].copy()
        for g in range(1, GROUPS):
            acc += res.results[GROUPS * b + g]["out"]
        out[b] = acc + bo
    return out


# revision 8
# speedup vs baseline: 1.1716x; 1.1716x over previous
"""Multi-head attention (B=2, S=2048, HID=1024, NH=16, DH=64) on 8 trn2 cores.

Sharding: tensor-parallel over (batch, head-group): core c handles batch c//4
and heads 4*(c%4)..4*(c%4)+3 (256 of the 1024 hidden dims). Each core computes
q/k/v projections for its heads, attention, and a partial output projection;
the host sums the 4 partials per batch and adds the output bias.

Layout strategy (matmul operands are fp16 — full PE clock, fast weight load,
fp32 PSUM accumulation; fp32->fp16 input rounding costs ~5e-4 relative error):
  - Host pre-transposes Q/K/V ([HID, S] per batch) and weights so no on-device
    transposes are needed.
  - qT, kT are kept head-major [dh, S]; scores are computed transposed
    (sT[key, query] = kT.T @ qT) with two heads packed into the 128-wide PE
    contraction via row-group tile_position.
  - exp(sT) tiles feed ctxT = v_aug.T @ expT where v_aug has a ones column, so
    the softmax denominator Z accumulates in PSUM row 64 for free.
  - The reference's mask quirk (masked scores set to -1e-9, NOT -inf) makes a
    masked entry contribute exp(-1e-9) == 1.0f; softmax max-subtraction is
    skipped (scores are O(1), exp is safe) so masked entries are exactly 1.0.
    Future-key blocks are therefore never computed: their contribution is a
    rank-1 update (suffix-sums of v_aug) added straight into the ctx PSUM.
"""

import numpy as np

B, S, HID, NH, DH = 2, 2048, 1024, 16, 64
NCORES = 8
GROUPS = 4            # head groups (cores per batch)
HPC = NH // GROUPS    # 4 heads per core
HDS = HPC * DH        # 256 hidden dims per core
QW = 512              # query-chunk width (one fp32 PSUM bank)
NCQ = S // QW         # 4 query chunks
NKT = S // 128        # 16 key tiles

_progs = {}


def _split_sync_waits(nc, max_waits: int = 1) -> int:
    """neuronxcc walrus codegen rejects instructions with more than one sync
    wait ("Too many sync wait commands"). Move excess waits onto preceding
    same-engine NoOps."""
    import concourse.mybir as mybir

    n_split = 0
    for fn in nc.m.functions:
        for bb in fn.blocks:
            out = []
            for ins in bb.instructions:
                si = ins.sync_info
                if si is not None and si.on_wait and len(si.on_wait) > max_waits:
                    waits = list(si.on_wait)
                    extra, keep = waits[:-max_waits], waits[-max_waits:]
                    for i in range(0, len(extra), max_waits):
                        chunk = extra[i:i + max_waits]
                        nop = mybir.InstNoOp(
                            name=nc.get_next_instruction_name(),
                            engine=ins.engine,
                            ins=[],
                            outs=[],
                            sync_info=mybir.SyncInfo(on_wait=chunk, on_update=[]),
                            bass_nofuse=True,
                            text_hint="split_sync_waits",
                        )
                        out.append(nop)
                        n_split += 1
                    si.on_wait = keep
                out.append(ins)
            bb.instructions[:] = out
    return n_split


def _build_program(causal: bool):
    import concourse.bass as bass
    import concourse.tile as tile
    from concourse import mybir

    f32 = mybir.dt.float32
    f16 = mybir.dt.float16
    Ident = mybir.ActivationFunctionType.Identity
    Copy = mybir.ActivationFunctionType.Copy
    Exp = mybir.ActivationFunctionType.Exp

    nc = bass.Bass()
    qt_d = nc.dram_tensor("qt", [HID, S], f16, kind="ExternalInput")
    kt_d = nc.dram_tensor("kt", [HID, S], f16, kind="ExternalInput")
    vt_d = nc.dram_tensor("vt", [HID, S], f16, kind="ExternalInput")
    wq_d = nc.dram_tensor("wq", [HID, HDS], f16, kind="ExternalInput")
    wk_d = nc.dram_tensor("wk", [HID, HDS], f16, kind="ExternalInput")
    wv_d = nc.dram_tensor("wv", [HID, HDS], f16, kind="ExternalInput")
    wo_d = nc.dram_tensor("wo", [HDS, HID], f16, kind="ExternalInput")
    bq_d = nc.dram_tensor("bq", [128, 2], f32, kind="ExternalInput")
    bk_d = nc.dram_tensor("bk", [128, 2], f32, kind="ExternalInput")
    bv_d = nc.dram_tensor("bv", [1, HDS], f16, kind="ExternalInput")
    on_d = nc.dram_tensor("on", [128, QW], f16, kind="ExternalInput")
    mk_d = nc.dram_tensor("mk", [128, 128], f32, kind="ExternalInput")
    out_d = nc.dram_tensor("out", [S, HID], f32, kind="ExternalOutput")

    qt_r = qt_d.rearrange("(ko p) s -> p ko s", p=128)
    kt_r = kt_d.rearrange("(ko p) s -> p ko s", p=128)
    vt_r = vt_d.rearrange("(ko p) s -> p ko s", p=128)

    with tile.TileContext(nc) as tc:
        with tc.tile_pool(name="persist", bufs=1) as persist:
            qT = persist.tile([128, 2, S], f16)       # [dh(2 heads), m, s]
            kT = persist.tile([128, 2, S], f16)
            vA = persist.tile([128, NKT, HPC, DH + 1], f16)  # v_aug per s-tile
            cT = persist.tile([128, 2, S], f16)       # ctxT (divided by Z)
            ones = persist.tile([128, QW], f16)
            maskt = persist.tile([128, 128], f32)
            bq_sb = persist.tile([128, 2], f32)
            bk_sb = persist.tile([128, 2], f32)
            bv_sb = persist.tile([1, HDS], f16)
            vs_sb = persist.tile([1, 3, HPC * (DH + 1)], f16)
            wo_sb = persist.tile([128, 2, HID], f16)

            nc.sync.dma_start(ones[:], on_d[:])
            nc.sync.dma_start(maskt[:], mk_d[:])
            nc.sync.dma_start(bq_sb[:], bq_d[:])
            nc.sync.dma_start(bk_sb[:], bk_d[:])
            nc.sync.dma_start(bv_sb[:], bv_d[:])
            nc.sync.dma_start(wo_sb[:], wo_d.rearrange("(ko p) o -> p ko o", p=128))

            # ---------- Phase A: projections ----------
            with (
                tc.tile_pool(name="wpool", bufs=1) as wpool,
                tc.tile_pool(name="app", bufs=4, space="PSUM") as app,
                tc.tile_pool(name="vpp", bufs=2, space="PSUM") as vpp,
                tc.tile_pool(name="arhs", bufs=2) as arhs,
                tc.tile_pool(name="vsl", bufs=3) as vsl,
            ):
                wq_sb = wpool.tile([128, 8, HDS], f16, tag="wq")
                wk_sb = wpool.tile([128, 8, HDS], f16, tag="wk")
                wv_sb = wpool.tile([128, 8, HDS], f16, tag="wv")
                nc.sync.dma_start(wq_sb[:], wq_d.rearrange("(ko p) m -> p ko m", p=128))
                nc.sync.dma_start(wk_sb[:], wk_d.rearrange("(ko p) m -> p ko m", p=128))
                nc.sync.dma_start(wv_sb[:], wv_d.rearrange("(ko p) m -> p ko m", p=128))

                for src_r, w_sb, b_sb, dstT in (
                    (qt_r, wq_sb, bq_sb, qT),
                    (kt_r, wk_sb, bk_sb, kT),
                ):
                    for ns in range(4):
                        rh = arhs.tile([128, 8, QW], f16, tag="projrhs")
                        nc.sync.dma_start(rh[:], src_r[:, :, ns * QW:(ns + 1) * QW])
                        for m in range(2):
                            ps = app.tile([128, QW], f32, tag="projps")
                            for ko in range(8):
                                nc.tensor.matmul(
                                    ps[:],
                                    w_sb[:, ko, m * 128:(m + 1) * 128],
                                    rh[:, ko, :],
                                    start=(ko == 0),
                                    stop=(ko == 7),
                                )
                            nc.scalar.activation(
                                dstT[:, m, ns * QW:(ns + 1) * QW], ps[:],
                                Ident, bias=b_sb[:, m:m + 1],
                            )

                # ones column of v_aug (written once, before per-tile evacs)
                nc.scalar.activation(
                    vA[:, :, :, DH:DH + 1],
                    ones[:, 0:NKT * HPC].rearrange("p (a b o) -> p a b o", a=NKT, b=HPC),
                    Copy,
                )
                for st in range(NKT):
                    vslab = vsl.tile([128, 8, 128], f16, tag="vslab")
                    nc.sync.dma_start(vslab[:], vt_r[:, :, st * 128:(st + 1) * 128])
                    ps = vpp.tile([128, HDS], f32, tag="vps")
                    for ko in range(8):
                        nc.tensor.matmul(
                            ps[:], vslab[:, ko, :], wv_sb[:, ko, :],
                            start=(ko == 0), stop=False,
                        )
                    nc.tensor.matmul(
                        ps[:], ones[0:1, 0:128], bv_sb[0:1, :],
                        start=False, stop=True,
                    )
                    nc.scalar.activation(
                        vA[:, st, :, 0:DH],
                        ps.rearrange("p (h d) -> p h d", h=HPC),
                        Ident,
                    )

                if causal:
                    # suffix sums of v_aug column-totals: vs_sb[0, c-1, :] =
                    # sum_{st >= 4c} colsum(v_aug[st])  (includes key counts)
                    for c in (1, 2, 3):
                        vps = vpp.tile([1, HPC * (DH + 1)], f32, tag="vsps")
                        for st in range(4 * c, NKT):
                            nc.tensor.matmul(
                                vps[:],
                                ones[:, 0:1],
                                vA[:, st, :, :].rearrange("p a b -> p (a b)"),
                                start=(st == 4 * c),
                                stop=(st == NKT - 1),
                            )
                        nc.vector.tensor_copy(vs_sb[0:1, c - 1, :], vps[:])

            # ---------- Phase B: attention ----------
            with (
                tc.tile_pool(name="sps", bufs=4, space="PSUM") as sps,
                tc.tile_pool(name="cps", bufs=2, space="PSUM") as cps,
                tc.tile_pool(name="zps", bufs=2, space="PSUM") as zps,
                tc.tile_pool(name="esb", bufs=4) as esb,
                tc.tile_pool(name="fin", bufs=2) as fin,
            ):
                for hp in range(2):
                    for cq in range(NCQ):
                        n_kt = 4 * (cq + 1) if causal else NKT
                        last_on_loop = (not causal) or cq == 3
                        ctx = [
                            cps.tile([DH + 1, QW], f32, tag="ctx", name=f"ctx{i}")
                            for i in range(2)
                        ]
                        for kt_i in range(n_kt):
                            es = []
                            for hl in range(2):
                                lo = 64 * hl
                                r = kt_i - 4 * cq
                                pre = r * 128 if (causal and r >= 1) else 0
                                s_ps = sps.tile([128, QW], f32, tag="s")
                                nc.tensor.matmul(
                                    s_ps[:, pre:],
                                    kT[lo:lo + 64, hp, kt_i * 128:(kt_i + 1) * 128],
                                    qT[lo:lo + 64, hp, cq * QW + pre:(cq + 1) * QW],
                                    start=True, stop=True,
                                    tile_position=(lo, 0),
                                )
                                if causal and r >= 0:
                                    nc.vector.tensor_mul(
                                        s_ps[:, r * 128:(r + 1) * 128],
                                        s_ps[:, r * 128:(r + 1) * 128],
                                        maskt[:],
                                    )
                                e = esb.tile([128, QW], f16, tag="e")
                                nc.scalar.activation(
                                    e[:, pre:], s_ps[:, pre:], Exp, scale=0.125
                                )
                                if pre:
                                    nc.vector.tensor_copy(
                                        e[:, 0:pre], ones[:, 0:pre]
                                    )
                                es.append(e)
                            for hl in range(2):
                                h = 2 * hp + hl
                                nc.tensor.matmul(
                                    ctx[hl][:],
                                    vA[:, kt_i, h, :],
                                    es[hl][:],
                                    start=(kt_i == 0),
                                    stop=(kt_i == n_kt - 1 and last_on_loop),
                                )
                        if causal and cq < 3:
                            for hl in range(2):
                                h = 2 * hp + hl
                                nc.tensor.matmul(
                                    ctx[hl][:],
                                    vs_sb[0:1, cq, (DH + 1) * h:(DH + 1) * (h + 1)],
                                    ones[0:1, 0:QW],
                                    start=False, stop=True,
                                )
                        for hl in range(2):
                            zr = fin.tile([DH + 1, QW], f16, tag="zr")
                            with nc.allow_low_precision(reason="fp16 recip"):
                                nc.vector.reciprocal(
                                    zr[DH:DH + 1, :], ctx[hl][DH:DH + 1, :]
                                )
                            zb = zps.tile([DH, QW], f32, tag="zb")
                            nc.tensor.matmul(
                                zb[:], ones[64:65, 0:DH], zr[DH:DH + 1, :],
                                start=True, stop=True,
                            )
                            zc = fin.tile([DH, QW], f32, tag="zc")
                            nc.vector.tensor_copy(zc[:], zb[:])
                            nc.vector.tensor_mul(
                                cT[64 * hl:64 * (hl + 1), hp, cq * QW:(cq + 1) * QW],
                                ctx[hl][0:DH, :],
                                zc[:],
                            )

            # ---------- Phase C: output projection (partial) ----------
            with (
                tc.tile_pool(name="ops", bufs=4, space="PSUM") as ops_p,
                tc.tile_pool(name="osb", bufs=3) as osb,
            ):
                for q_i in range(NKT):
                    ost = osb.tile([128, 2, QW], f32, tag="ost")
                    for no in range(2):
                        ps = ops_p.tile([128, QW], f32, tag="ops")
                        for ko in range(2):
                            nc.tensor.matmul(
                                ps[:],
                                cT[:, ko, q_i * 128:(q_i + 1) * 128],
                                wo_sb[:, ko, no * QW:(no + 1) * QW],
                                start=(ko == 0), stop=(ko == 1),
                            )
                        nc.vector.tensor_copy(ost[:, no, :], ps[:])
                    nc.sync.dma_start(
                        out_d[q_i * 128:(q_i + 1) * 128, :],
                        ost.rearrange("p a b -> p (a b)"),
                    )

    _split_sync_waits(nc)
    return nc


def _get_program(causal: bool):
    if causal not in _progs:
        _progs[causal] = _build_program(causal)
    return _progs[causal]


def _numpy_fallback(Q, K, V, pad_mask, attn_mask, Wq, bq, Wk, bk, Wv, bv, Wo, bo):
    NEG = np.float32(-1e-09)

    def split_heads(x):
        return x.reshape(B, S, NH, DH).transpose(0, 2, 1, 3)

    q = split_heads(Q @ Wq.T + bq)
    k = split_heads(K @ Wk.T + bk)
    v = split_heads(V @ Wv.T + bv)
    scores = np.einsum("bhqd,bhkd->bhqk", q, k) / np.sqrt(DH)
    mask = pad_mask[:, :, None] * pad_mask[:, None, :] * attn_mask
    scores = np.where(mask[:, None, :, :] != 0, scores, NEG)
    scores = scores - scores.max(axis=-1, keepdims=True)
    e = np.exp(scores)
    attn = e / e.sum(axis=-1, keepdims=True)
    ctx = np.einsum("bhqk,bhkd->bhqd", attn, v)
    ctx = ctx.transpose(0, 2, 1, 3).reshape(B, S, HID)
    return (ctx @ Wo.T + bo).astype(np.float32)


def kernel(Q, K, V, pad_mask, attn_mask, Wq, bq, Wk, bk, Wv, bv, Wo, bo):
    Q = np.asarray(Q, np.float32)
    K = np.asarray(K, np.float32)
    V = np.asarray(V, np.float32)
    pad_mask = np.asarray(pad_mask, np.float32)
    attn_mask = np.asarray(attn_mask, np.float32)
    Wq = np.asarray(Wq, np.float32)
    bq = np.asarray(bq, np.float32)
    Wk = np.asarray(Wk, np.float32)
    bk = np.asarray(bk, np.float32)
    Wv = np.asarray(Wv, np.float32)
    bv = np.asarray(bv, np.float32)
    Wo = np.asarray(Wo, np.float32)
    bo = np.asarray(bo, np.float32)

    tril = np.tril(np.ones((S, S), np.float32))
    if not np.all(pad_mask == 1.0):
        return _numpy_fallback(Q, K, V, pad_mask, attn_mask,
                               Wq, bq, Wk, bk, Wv, bv, Wo, bo)
    if np.array_equal(attn_mask, tril):
        causal = True
    elif np.all(attn_mask != 0.0):
        causal = False
    else:
        return _numpy_fallback(Q, K, V, pad_mask, attn_mask,
                               Wq, bq, Wk, bk, Wv, bv, Wo, bo)

    from concourse.bass_utils import run_bass_kernel_spmd

    nc = _get_program(causal)

    ones = np.ones((128, QW), np.float16)
    maskt = np.triu(np.ones((128, 128), np.float32))  # [key, query]: key<=query
    WqT = np.ascontiguousarray(Wq.T.astype(np.float16))
    WkT = np.ascontiguousarray(Wk.T.astype(np.float16))
    WvT = np.ascontiguousarray(Wv.T.astype(np.float16))
    WoT = np.ascontiguousarray(Wo.T.astype(np.float16))
    QT = [np.ascontiguousarray(Q[b].T.astype(np.float16)) for b in range(B)]
    KT = [np.ascontiguousarray(K[b].T.astype(np.float16)) for b in range(B)]
    VT = [np.ascontiguousarray(V[b].T.astype(np.float16)) for b in range(B)]

    in_maps = []
    for c in range(NCORES):
        b, g = divmod(c, GROUPS)
        hs = slice(g * HDS, (g + 1) * HDS)
        in_maps.append({
            "qt": QT[b], "kt": KT[b], "vt": VT[b],
            "wq": np.ascontiguousarray(WqT[:, hs]),
            "wk": np.ascontiguousarray(WkT[:, hs]),
            "wv": np.ascontiguousarray(WvT[:, hs]),
            "wo": np.ascontiguousarray(WoT[hs, :]),
            "bq": np.ascontiguousarray(bq[hs].reshape(2, 128).T),
            "bk": np.ascontiguousarray(bk[hs].reshape(2, 128).T),
            "bv": np.ascontiguousarray(bv[hs].reshape(1, HDS).astype(np.float16)),
            "on": ones, "mk": maskt,
        })

    global _trace_in_maps
    _trace_in_maps = in_maps

    res = run_bass_kernel_spmd(nc, in_maps, core_ids=list(range(NCORES)))
    out = np.empty((B, S, HID), np.float32)
    for b in range(B):
        acc = res.results[GROUPS * b]["out"].copy()
        for g in range(1, GROUPS):
            acc += res.results[GROUPS * b + g]["out"]
        out[b] = acc + bo
    return out


# revision 10
# speedup vs baseline: 1.1902x; 1.0158x over previous
"""Multi-head attention (B=2, S=2048, HID=1024, NH=16, DH=64) on 8 trn2 cores.

Sharding: tensor-parallel over (batch, head-group): core c handles batch c//4
and heads 4*(c%4)..4*(c%4)+3 (256 of the 1024 hidden dims). Each core computes
q/k/v projections for its heads, attention, and a partial output projection;
the host sums the 4 partials per batch and adds the output bias.

Layout strategy (matmul operands are fp16 — full PE clock, fast weight load,
fp32 PSUM accumulation; fp32->fp16 input rounding costs ~5e-4 relative error):
  - Host pre-transposes Q/K/V ([HID, S] per batch) and weights so no on-device
    transposes are needed.
  - qT, kT are kept head-major [dh, S]; scores are computed transposed
    (sT[key, query] = kT.T @ qT) with two heads packed into the 128-wide PE
    contraction via row-group tile_position.
  - exp(sT) tiles feed ctxT = v_aug.T @ expT where v_aug has a ones column, so
    the softmax denominator Z accumulates in PSUM row 64 for free.
  - The reference's mask quirk (masked scores set to -1e-9, NOT -inf) makes a
    masked entry contribute exp(-1e-9) == 1.0f; softmax max-subtraction is
    skipped (scores are O(1), exp is safe) so masked entries are exactly 1.0.
    Future-key blocks are therefore never computed: their contribution is a
    rank-1 update (suffix-sums of v_aug) added straight into the ctx PSUM.
"""

import numpy as np

B, S, HID, NH, DH = 2, 2048, 1024, 16, 64
NCORES = 8
GROUPS = 4            # head groups (cores per batch)
HPC = NH // GROUPS    # 4 heads per core
HDS = HPC * DH        # 256 hidden dims per core
QW = 512              # query-chunk width (one fp32 PSUM bank)
NCQ = S // QW         # 4 query chunks
NKT = S // 128        # 16 key tiles

_progs = {}


def _split_sync_waits(nc, max_waits: int = 1) -> int:
    """neuronxcc walrus codegen rejects instructions with more than one sync
    wait ("Too many sync wait commands"). Move excess waits onto preceding
    same-engine NoOps."""
    import concourse.mybir as mybir

    n_split = 0
    for fn in nc.m.functions:
        for bb in fn.blocks:
            out = []
            for ins in bb.instructions:
                si = ins.sync_info
                if si is not None and si.on_wait and len(si.on_wait) > max_waits:
                    waits = list(si.on_wait)
                    extra, keep = waits[:-max_waits], waits[-max_waits:]
                    for i in range(0, len(extra), max_waits):
                        chunk = extra[i:i + max_waits]
                        nop = mybir.InstNoOp(
                            name=nc.get_next_instruction_name(),
                            engine=ins.engine,
                            ins=[],
                            outs=[],
                            sync_info=mybir.SyncInfo(on_wait=chunk, on_update=[]),
                            bass_nofuse=True,
                            text_hint="split_sync_waits",
                        )
                        out.append(nop)
                        n_split += 1
                    si.on_wait = keep
                out.append(ins)
            bb.instructions[:] = out
    return n_split


def _build_program(causal: bool):
    import concourse.bass as bass
    import concourse.tile as tile
    from concourse import mybir

    f32 = mybir.dt.float32
    f16 = mybir.dt.float16
    Ident = mybir.ActivationFunctionType.Identity
    Copy = mybir.ActivationFunctionType.Copy
    Exp = mybir.ActivationFunctionType.Exp

    nc = bass.Bass()
    qt_d = nc.dram_tensor("qt", [HID, S], f16, kind="ExternalInput")
    kt_d = nc.dram_tensor("kt", [HID, S], f16, kind="ExternalInput")
    vt_d = nc.dram_tensor("vt", [HID, S], f16, kind="ExternalInput")
    wq_d = nc.dram_tensor("wq", [HID, HDS], f16, kind="ExternalInput")
    wk_d = nc.dram_tensor("wk", [HID, HDS], f16, kind="ExternalInput")
    wv_d = nc.dram_tensor("wv", [HID, HDS], f16, kind="ExternalInput")
    wo_d = nc.dram_tensor("wo", [HDS, HID], f16, kind="ExternalInput")
    bq_d = nc.dram_tensor("bq", [128, 2], f32, kind="ExternalInput")
    bk_d = nc.dram_tensor("bk", [128, 2], f32, kind="ExternalInput")
    bv_d = nc.dram_tensor("bv", [1, HDS], f16, kind="ExternalInput")
    on_d = nc.dram_tensor("on", [128, QW], f16, kind="ExternalInput")
    mk_d = nc.dram_tensor("mk", [128, 128], f32, kind="ExternalInput")
    out_d = nc.dram_tensor("out", [S, HID], f32, kind="ExternalOutput")

    qt_r = qt_d.rearrange("(ko p) s -> p ko s", p=128)
    kt_r = kt_d.rearrange("(ko p) s -> p ko s", p=128)
    vt_r = vt_d.rearrange("(ko p) s -> p ko s", p=128)

    with tile.TileContext(nc) as tc:
        with tc.tile_pool(name="persist", bufs=1) as persist:
            qT = persist.tile([128, 2, S], f16)       # [dh(2 heads), m, s]
            kT = persist.tile([128, 2, S], f16)
            vA = persist.tile([128, NKT, HPC, 128], f16)  # v_aug (padded to M=128)
            cT = persist.tile([128, 2, S], f16)       # ctxT (divided by Z)
            ones = persist.tile([128, QW], f16)
            maskt = persist.tile([128, 128], f32)
            bq_sb = persist.tile([128, 2], f32)
            bk_sb = persist.tile([128, 2], f32)
            bv_sb = persist.tile([1, HDS], f16)
            vs_sb = persist.tile([1, 3, HPC * 128], f16)
            wo_sb = persist.tile([128, 2, HID], f16)

            nc.sync.dma_start(ones[:], on_d[:])
            nc.sync.dma_start(maskt[:], mk_d[:])
            nc.sync.dma_start(bq_sb[:], bq_d[:])
            nc.sync.dma_start(bk_sb[:], bk_d[:])
            nc.sync.dma_start(bv_sb[:], bv_d[:])
            nc.sync.dma_start(wo_sb[:], wo_d.rearrange("(ko p) o -> p ko o", p=128))

            # ---------- Phase A: projections ----------
            with (
                tc.tile_pool(name="wpool", bufs=1) as wpool,
                tc.tile_pool(name="app", bufs=4, space="PSUM") as app,
                tc.tile_pool(name="vpp", bufs=2, space="PSUM") as vpp,
                tc.tile_pool(name="arhs", bufs=2) as arhs,
                tc.tile_pool(name="vsl", bufs=3) as vsl,
            ):
                wq_sb = wpool.tile([128, 8, HDS], f16, tag="wq")
                wk_sb = wpool.tile([128, 8, HDS], f16, tag="wk")
                wv_sb = wpool.tile([128, 8, HDS], f16, tag="wv")
                nc.sync.dma_start(wq_sb[:], wq_d.rearrange("(ko p) m -> p ko m", p=128))
                nc.sync.dma_start(wk_sb[:], wk_d.rearrange("(ko p) m -> p ko m", p=128))
                nc.sync.dma_start(wv_sb[:], wv_d.rearrange("(ko p) m -> p ko m", p=128))

                for src_r, w_sb, b_sb, dstT in (
                    (qt_r, wq_sb, bq_sb, qT),
                    (kt_r, wk_sb, bk_sb, kT),
                ):
                    for ns in range(4):
                        rh = arhs.tile([128, 8, QW], f16, tag="projrhs")
                        nc.sync.dma_start(rh[:], src_r[:, :, ns * QW:(ns + 1) * QW])
                        for m in range(2):
                            ps = app.tile([128, QW], f32, tag="projps")
                            for ko in range(8):
                                nc.tensor.matmul(
                                    ps[:],
                                    w_sb[:, ko, m * 128:(m + 1) * 128],
                                    rh[:, ko, :],
                                    start=(ko == 0),
                                    stop=(ko == 7),
                                )
                            nc.scalar.activation(
                                dstT[:, m, ns * QW:(ns + 1) * QW], ps[:],
                                Ident, bias=b_sb[:, m:m + 1],
                            )

                # zero pad columns + ones column (written once, before evacs)
                nc.vector.memset(vA[:, :, :, DH + 1:128], 0.0)
                nc.scalar.activation(
                    vA[:, :, :, DH:DH + 1],
                    ones[:, 0:NKT * HPC].rearrange("p (a b o) -> p a b o", a=NKT, b=HPC),
                    Copy,
                )
                for st in range(NKT):
                    vslab = vsl.tile([128, 8, 128], f16, tag="vslab")
                    nc.sync.dma_start(vslab[:], vt_r[:, :, st * 128:(st + 1) * 128])
                    ps = vpp.tile([128, HDS], f32, tag="vps")
                    for ko in range(8):
                        nc.tensor.matmul(
                            ps[:], vslab[:, ko, :], wv_sb[:, ko, :],
                            start=(ko == 0), stop=False,
                        )
                    nc.tensor.matmul(
                        ps[:], ones[0:1, 0:128], bv_sb[0:1, :],
                        start=False, stop=True,
                    )
                    nc.scalar.activation(
                        vA[:, st, :, 0:DH],
                        ps.rearrange("p (h d) -> p h d", h=HPC),
                        Ident,
                    )

                if causal:
                    # suffix sums of v_aug column-totals: vs_sb[0, c-1, :] =
                    # sum_{st >= 4c} colsum(v_aug[st])  (includes key counts)
                    for c in (1, 2, 3):
                        vps = vpp.tile([1, HPC * 128], f32, tag="vsps")
                        for st in range(4 * c, NKT):
                            nc.tensor.matmul(
                                vps[:],
                                ones[:, 0:1],
                                vA[:, st, :, :].rearrange("p a b -> p (a b)"),
                                start=(st == 4 * c),
                                stop=(st == NKT - 1),
                            )
                        nc.vector.tensor_copy(vs_sb[0:1, c - 1, :], vps[:])

            # ---------- Phase B: attention ----------
            with (
                tc.tile_pool(name="sps", bufs=4, space="PSUM") as sps,
                tc.tile_pool(name="cps", bufs=2, space="PSUM") as cps,
                tc.tile_pool(name="zps", bufs=2, space="PSUM") as zps,
                tc.tile_pool(name="esb", bufs=4) as esb,
                tc.tile_pool(name="fin", bufs=2) as fin,
            ):
                for hp in range(2):
                    for cq in range(NCQ):
                        n_kt = 4 * (cq + 1) if causal else NKT
                        last_on_loop = (not causal) or cq == 3
                        ctx = [
                            cps.tile([128, QW], f32, tag="ctx", name=f"ctx{i}")
                            for i in range(2)
                        ]
                        for kt_i in range(n_kt):
                            es = []
                            for hl in range(2):
                                lo = 64 * hl
                                r = kt_i - 4 * cq
                                pre = r * 128 if (causal and r >= 1) else 0
                                s_ps = sps.tile([128, QW], f32, tag="s")
                                nc.tensor.matmul(
                                    s_ps[:, pre:],
                                    kT[lo:lo + 64, hp, kt_i * 128:(kt_i + 1) * 128],
                                    qT[lo:lo + 64, hp, cq * QW + pre:(cq + 1) * QW],
                                    start=True, stop=True,
                                    tile_position=(lo, 0),
                                )
                                if causal and r >= 0:
                                    nc.vector.tensor_mul(
                                        s_ps[:, r * 128:(r + 1) * 128],
                                        s_ps[:, r * 128:(r + 1) * 128],
                                        maskt[:],
                                    )
                                e = esb.tile([128, QW], f16, tag="e")
                                nc.scalar.activation(
                                    e[:, pre:], s_ps[:, pre:], Exp, scale=0.125
                                )
                                if pre:
                                    nc.gpsimd.tensor_copy(
                                        e[:, 0:pre], ones[:, 0:pre]
                                    )
                                es.append(e)
                            for hl in range(2):
                                h = 2 * hp + hl
                                nc.tensor.matmul(
                                    ctx[hl][:],
                                    vA[:, kt_i, h, :],
                                    es[hl][:],
                                    start=(kt_i == 0),
                                    stop=(kt_i == n_kt - 1 and last_on_loop),
                                )
                        if causal and cq < 3:
                            for hl in range(2):
                                h = 2 * hp + hl
                                nc.tensor.matmul(
                                    ctx[hl][0:DH + 1, :],
                                    vs_sb[0:1, cq, 128 * h:128 * h + DH + 1],
                                    ones[0:1, 0:QW],
                                    start=False, stop=True,
                                )
                        for hl in range(2):
                            zr = fin.tile([DH + 1, QW], f16, tag="zr")
                            with nc.allow_low_precision(reason="fp16 recip"):
                                nc.vector.reciprocal(
                                    zr[DH:DH + 1, :], ctx[hl][DH:DH + 1, :]
                                )
                            zb = zps.tile([DH, QW], f32, tag="zb")
                            nc.tensor.matmul(
                                zb[:], ones[64:65, 0:DH], zr[DH:DH + 1, :],
                                start=True, stop=True,
                            )
                            zc = fin.tile([DH, QW], f32, tag="zc")
                            nc.vector.tensor_copy(zc[:], zb[:])
                            nc.vector.tensor_mul(
                                cT[64 * hl:64 * (hl + 1), hp, cq * QW:(cq + 1) * QW],
                                ctx[hl][0:DH, :],
                                zc[:],
                            )

            # ---------- Phase C: output projection (partial) ----------
            with (
                tc.tile_pool(name="ops", bufs=4, space="PSUM") as ops_p,
                tc.tile_pool(name="osb", bufs=3) as osb,
            ):
                for q_i in range(NKT):
                    ost = osb.tile([128, 2, QW], f32, tag="ost")
                    for no in range(2):
                        ps = ops_p.tile([128, QW], f32, tag="ops")
                        for ko in range(2):
                            nc.tensor.matmul(
                                ps[:],
                                cT[:, ko, q_i * 128:(q_i + 1) * 128],
                                wo_sb[:, ko, no * QW:(no + 1) * QW],
                                start=(ko == 0), stop=(ko == 1),
                            )
                        nc.vector.tensor_copy(ost[:, no, :], ps[:])
                    nc.sync.dma_start(
                        out_d[q_i * 128:(q_i + 1) * 128, :],
                        ost.rearrange("p a b -> p (a b)"),
                    )

    _split_sync_waits(nc)
    return nc


def _get_program(causal: bool):
    if causal not in _progs:
        _progs[causal] = _build_program(causal)
    return _progs[causal]


def _numpy_fallback(Q, K, V, pad_mask, attn_mask, Wq, bq, Wk, bk, Wv, bv, Wo, bo):
    NEG = np.float32(-1e-09)

    def split_heads(x):
        return x.reshape(B, S, NH, DH).transpose(0, 2, 1, 3)

    q = split_heads(Q @ Wq.T + bq)
    k = split_heads(K @ Wk.T + bk)
    v = split_heads(V @ Wv.T + bv)
    scores = np.einsum("bhqd,bhkd->bhqk", q, k) / np.sqrt(DH)
    mask = pad_mask[:, :, None] * pad_mask[:, None, :] * attn_mask
    scores = np.where(mask[:, None, :, :] != 0, scores, NEG)
    scores = scores - scores.max(axis=-1, keepdims=True)
    e = np.exp(scores)
    attn = e / e.sum(axis=-1, keepdims=True)
    ctx = np.einsum("bhqk,bhkd->bhqd", attn, v)
    ctx = ctx.transpose(0, 2, 1, 3).reshape(B, S, HID)
    return (ctx @ Wo.T + bo).astype(np.float32)


def kernel(Q, K, V, pad_mask, attn_mask, Wq, bq, Wk, bk, Wv, bv, Wo, bo):
    Q = np.asarray(Q, np.float32)
    K = np.asarray(K, np.float32)
    V = np.asarray(V, np.float32)
    pad_mask = np.asarray(pad_mask, np.float32)
    attn_mask = np.asarray(attn_mask, np.float32)
    Wq = np.asarray(Wq, np.float32)
    bq = np.asarray(bq, np.float32)
    Wk = np.asarray(Wk, np.float32)
    bk = np.asarray(bk, np.float32)
    Wv = np.asarray(Wv, np.float32)
    bv = np.asarray(bv, np.float32)
    Wo = np.asarray(Wo, np.float32)
    bo = np.asarray(bo, np.float32)

    tril = np.tril(np.ones((S, S), np.float32))
    if not np.all(pad_mask == 1.0):
        return _numpy_fallback(Q, K, V, pad_mask, attn_mask,
                               Wq, bq, Wk, bk, Wv, bv, Wo, bo)
    if np.array_equal(attn_mask, tril):
        causal = True
    elif np.all(attn_mask != 0.0):
        causal = False
    else:
        return _numpy_fallback(Q, K, V, pad_mask, attn_mask,
                               Wq, bq, Wk, bk, Wv, bv, Wo, bo)

    from concourse.bass_utils import run_bass_kernel_spmd

    nc = _get_program(causal)

    ones = np.ones((128, QW), np.float16)
    maskt = np.triu(np.ones((128, 128), np.float32))  # [key, query]: key<=query
    WqT = np.ascontiguousarray(Wq.T.astype(np.float16))
    WkT = np.ascontiguousarray(Wk.T.astype(np.float16))
    WvT = np.ascontiguousarray(Wv.T.astype(np.float16))
    WoT = np.ascontiguousarray(Wo.T.astype(np.float16))
    QT = [np.ascontiguousarray(Q[b].T.astype(np.float16)) for b in range(B)]
    KT = [np.ascontiguousarray(K[b].T.astype(np.float16)) for b in range(B)]
    VT = [np.ascontiguousarray(V[b].T.astype(np.float16)) for b in range(B)]

    in_maps = []
    for c in range(NCORES):
        b, g = divmod(c, GROUPS)
        hs = slice(g * HDS, (g + 1) * HDS)
        in_maps.append({
            "qt": QT[b], "kt": KT[b], "vt": VT[b],
            "wq": np.ascontiguousarray(WqT[:, hs]),
            "wk": np.ascontiguousarray(WkT[:, hs]),
            "wv": np.ascontiguousarray(WvT[:, hs]),
            "wo": np.ascontiguousarray(WoT[hs, :]),
            "bq": np.ascontiguousarray(bq[hs].reshape(2, 128).T),
            "bk": np.ascontiguousarray(bk[hs].reshape(2, 128).T),
            "bv": np.ascontiguousarray(bv[hs].reshape(1, HDS).astype(np.float16)),
            "on": ones, "mk": maskt,
        })

    global _trace_in_maps
    _trace_in_maps = in_maps

    res = run_bass_kernel_spmd(nc, in_maps, core_ids=list(range(NCORES)))
    out = np.empty((B, S, HID), np.float32)
    for b in range(B):
        acc = res.results[GROUPS * b]["out"].copy()
        for g in range(1, GROUPS):
            acc += res.results[GROUPS * b + g]["out"]
        out[b] = acc + bo
    return out


# revision 14
# speedup vs baseline: 1.3195x; 1.1087x over previous
"""Multi-head attention (B=2, S=2048, HID=1024, NH=16, DH=64) on 8 trn2 cores.

Sharding: tensor-parallel over (batch, head-group): core c handles batch c//4
and heads 4*(c%4)..4*(c%4)+3 (256 of the 1024 hidden dims). Each core computes
q/k/v projections for its heads, attention, and a partial output projection;
the host sums the 4 partials per batch and adds the output bias.

Layout strategy (matmul operands are fp16 — full PE clock, fast weight load,
fp32 PSUM accumulation; fp32->fp16 input rounding costs ~5e-4 relative error):
  - Host pre-transposes Q/K/V ([HID, S] per batch) and weights so no on-device
    transposes are needed.
  - qT, kT are kept head-major [dh, S]; scores are computed transposed
    (sT[key, query] = kT.T @ qT) with two heads packed into the 128-wide PE
    contraction via row-group tile_position.
  - exp(sT) tiles feed ctxT = v_aug.T @ expT where v_aug has a ones column, so
    the softmax denominator Z accumulates in PSUM row 64 for free.
  - The reference's mask quirk (masked scores set to -1e-9, NOT -inf) makes a
    masked entry contribute exp(-1e-9) == 1.0f; softmax max-subtraction is
    skipped (scores are O(1), exp is safe) so masked entries are exactly 1.0.
    Future-key blocks are therefore never computed: their contribution is a
    rank-1 update (suffix-sums of v_aug) added straight into the ctx PSUM.
"""

import numpy as np

B, S, HID, NH, DH = 2, 2048, 1024, 16, 64
NCORES = 8
GROUPS = 4            # head groups (cores per batch)
HPC = NH // GROUPS    # 4 heads per core
HDS = HPC * DH        # 256 hidden dims per core
QW = 512              # query-chunk width (one fp32 PSUM bank)
NCQ = S // QW         # 4 query chunks
NKT = S // 128        # 16 key tiles

_progs = {}


def _split_sync_waits(nc, max_waits: int = 1) -> int:
    """neuronxcc walrus codegen rejects instructions with more than one sync
    wait ("Too many sync wait commands"). Move excess waits onto preceding
    same-engine NoOps."""
    import concourse.mybir as mybir

    n_split = 0
    for fn in nc.m.functions:
        for bb in fn.blocks:
            out = []
            for ins in bb.instructions:
                si = ins.sync_info
                if si is not None and si.on_wait and len(si.on_wait) > max_waits:
                    waits = list(si.on_wait)
                    extra, keep = waits[:-max_waits], waits[-max_waits:]
                    for i in range(0, len(extra), max_waits):
                        chunk = extra[i:i + max_waits]
                        nop = mybir.InstNoOp(
                            name=nc.get_next_instruction_name(),
                            engine=ins.engine,
                            ins=[],
                            outs=[],
                            sync_info=mybir.SyncInfo(on_wait=chunk, on_update=[]),
                            bass_nofuse=True,
                            text_hint="split_sync_waits",
                        )
                        out.append(nop)
                        n_split += 1
                    si.on_wait = keep
                out.append(ins)
            bb.instructions[:] = out
    return n_split


def _build_program(causal: bool):
    import concourse.bass as bass
    import concourse.tile as tile
    from concourse import mybir

    f32 = mybir.dt.float32
    f16 = mybir.dt.float16
    Ident = mybir.ActivationFunctionType.Identity
    Copy = mybir.ActivationFunctionType.Copy
    Exp = mybir.ActivationFunctionType.Exp

    nc = bass.Bass()
    qt_d = nc.dram_tensor("qt", [HID, S], f16, kind="ExternalInput")
    kt_d = nc.dram_tensor("kt", [HID, S], f16, kind="ExternalInput")
    vt_d = nc.dram_tensor("vt", [HID, S], f16, kind="ExternalInput")
    wq_d = nc.dram_tensor("wq", [HID, HDS], f16, kind="ExternalInput")
    wk_d = nc.dram_tensor("wk", [HID, HDS], f16, kind="ExternalInput")
    wv_d = nc.dram_tensor("wv", [HID, HDS], f16, kind="ExternalInput")
    wo_d = nc.dram_tensor("wo", [HDS, HID], f16, kind="ExternalInput")
    bq_d = nc.dram_tensor("bq", [128, 2], f32, kind="ExternalInput")
    bk_d = nc.dram_tensor("bk", [128, 2], f32, kind="ExternalInput")
    bv_d = nc.dram_tensor("bv", [1, HDS], f16, kind="ExternalInput")
    on_d = nc.dram_tensor("on", [128, 2 * QW], f16, kind="ExternalInput")
    mk_d = nc.dram_tensor("mk", [128, 128], f32, kind="ExternalInput")
    out_d = nc.dram_tensor("out", [S, HID], f32, kind="ExternalOutput")

    qt_r = qt_d.rearrange("(ko p) s -> p ko s", p=128)
    kt_r = kt_d.rearrange("(ko p) s -> p ko s", p=128)
    vt_r = vt_d.rearrange("(ko p) s -> p ko s", p=128)

    with tile.TileContext(nc) as tc:
        with tc.tile_pool(name="persist", bufs=1) as persist:
            qT = persist.tile([128, 2, S], f16)       # [dh(2 heads), m, s]
            kT = persist.tile([128, 2, S], f16)
            vA = persist.tile([128, NKT, HPC, 128], f16)  # v_aug (padded to M=128)
            cT = persist.tile([128, 2, S], f16)       # ctxT (divided by Z)
            cN = persist.tile([128, 2, S], f16)       # ctxT numerator
            ones = persist.tile([128, 2 * QW], f16)
            maskt = persist.tile([128, 128], f32)
            bq_sb = persist.tile([128, 2], f32)
            bk_sb = persist.tile([128, 2], f32)
            bv_sb = persist.tile([1, HDS], f16)
            vs_sb = persist.tile([1, 3, HPC * 128], f16)
            wo_sb = persist.tile([128, 2, HID], f16)

            nc.sync.dma_start(ones[:], on_d[:])
            nc.sync.dma_start(maskt[:], mk_d[:])
            nc.sync.dma_start(bq_sb[:], bq_d[:])
            nc.sync.dma_start(bk_sb[:], bk_d[:])
            nc.sync.dma_start(bv_sb[:], bv_d[:])
            nc.sync.dma_start(wo_sb[:], wo_d.rearrange("(ko p) o -> p ko o", p=128))

            # ---------- Phase A: projections ----------
            with (
                tc.tile_pool(name="wpool", bufs=1) as wpool,
                tc.tile_pool(name="app", bufs=4, space="PSUM") as app,
                tc.tile_pool(name="vpp", bufs=2, space="PSUM") as vpp,
                tc.tile_pool(name="arhs", bufs=2) as arhs,
                tc.tile_pool(name="vsl", bufs=3) as vsl,
            ):
                wq_sb = wpool.tile([128, 8, HDS], f16, tag="wq")
                wk_sb = wpool.tile([128, 8, HDS], f16, tag="wk")
                wv_sb = wpool.tile([128, 8, HDS], f16, tag="wv")
                nc.sync.dma_start(wq_sb[:], wq_d.rearrange("(ko p) m -> p ko m", p=128))
                nc.sync.dma_start(wk_sb[:], wk_d.rearrange("(ko p) m -> p ko m", p=128))
                nc.sync.dma_start(wv_sb[:], wv_d.rearrange("(ko p) m -> p ko m", p=128))

                for src_r, w_sb, b_sb, dstT in (
                    (qt_r, wq_sb, bq_sb, qT),
                    (kt_r, wk_sb, bk_sb, kT),
                ):
                    for ns in range(4):
                        rh = arhs.tile([128, 8, QW], f16, tag="projrhs")
                        nc.sync.dma_start(rh[:], src_r[:, :, ns * QW:(ns + 1) * QW])
                        for m in range(2):
                            ps = app.tile([128, QW], f32, tag="projps")
                            for ko in range(8):
                                nc.tensor.matmul(
                                    ps[:],
                                    w_sb[:, ko, m * 128:(m + 1) * 128],
                                    rh[:, ko, :],
                                    start=(ko == 0),
                                    stop=(ko == 7),
                                )
                            nc.scalar.activation(
                                dstT[:, m, ns * QW:(ns + 1) * QW], ps[:],
                                Ident, bias=b_sb[:, m:m + 1],
                            )

                # zero pad columns + ones column (written once, before evacs)
                nc.vector.memset(vA[:, :, :, DH + 1:128], 0.0)
                nc.scalar.activation(
                    vA[:, :, :, DH:DH + 1],
                    ones[:, 0:NKT * HPC].rearrange("p (a b o) -> p a b o", a=NKT, b=HPC),
                    Copy,
                )
                for st in range(NKT):
                    vslab = vsl.tile([128, 8, 128], f16, tag="vslab")
                    nc.sync.dma_start(vslab[:], vt_r[:, :, st * 128:(st + 1) * 128])
                    ps = vpp.tile([128, HDS], f32, tag="vps")
                    for ko in range(8):
                        nc.tensor.matmul(
                            ps[:], vslab[:, ko, :], wv_sb[:, ko, :],
                            start=(ko == 0), stop=False,
                        )
                    nc.tensor.matmul(
                        ps[:], ones[0:1, 0:128], bv_sb[0:1, :],
                        start=False, stop=True,
                    )
                    nc.scalar.activation(
                        vA[:, st, :, 0:DH],
                        ps.rearrange("p (h d) -> p h d", h=HPC),
                        Ident,
                    )

                if causal:
                    # suffix sums of v_aug column-totals: vs_sb[0, c-1, :] =
                    # sum_{st >= 4c} colsum(v_aug[st])  (includes key counts)
                    for c in (1, 2, 3):
                        vps = vpp.tile([1, HPC * 128], f32, tag="vsps")
                        for st in range(4 * c, NKT):
                            nc.tensor.matmul(
                                vps[:],
                                ones[:, 0:1],
                                vA[:, st, :, :].rearrange("p a b -> p (a b)"),
                                start=(st == 4 * c),
                                stop=(st == NKT - 1),
                            )
                        nc.vector.tensor_copy(vs_sb[0:1, c - 1, :], vps[:])

            # ---------- Phase B: attention ----------
            # Z rows for group j (= 2*hp + cq//2) live at 32-aligned
            # partitions of zall[j] so a K=1 PE matmul can broadcast the
            # batched reciprocal back over the queries.
            zall = [persist.tile([97, QW], f32, name=f"zall{j}") for j in range(4)]
            rzt = [persist.tile([97, QW], f16, name=f"rzt{j}") for j in range(4)]
            with (
                tc.tile_pool(name="sps", bufs=2, space="PSUM") as sps,
                tc.tile_pool(name="cps", bufs=2, space="PSUM") as cps,
                tc.tile_pool(name="zps", bufs=2, space="PSUM") as zps,
                tc.tile_pool(name="esb", bufs=4) as esb,
            ):
                for hp in range(2):
                    for cq in range(NCQ):
                        n_kt = 4 * (cq + 1) if causal else NKT
                        last_on_loop = (not causal) or cq == 3
                        ctx = [
                            cps.tile([128, QW], f32, tag="ctx", name=f"ctx{i}")
                            for i in range(2)
                        ]
                        for kt_i in range(n_kt):
                            r = kt_i - 4 * cq
                            pre = r * 128 if (causal and r >= 1) else 0
                            s2 = sps.tile([128, 2, QW], f32, tag="s2")
                            for hl in range(2):
                                lo = 64 * hl
                                nc.tensor.matmul(
                                    s2[:, hl, pre:],
                                    kT[lo:lo + 64, hp, kt_i * 128:(kt_i + 1) * 128],
                                    qT[lo:lo + 64, hp, cq * QW + pre:(cq + 1) * QW],
                                    start=True, stop=True,
                                    tile_position=(lo, 0),
                                )
                            if causal and r >= 0:
                                for hl in range(2):
                                    nc.vector.tensor_mul(
                                        s2[:, hl, r * 128:(r + 1) * 128],
                                        s2[:, hl, r * 128:(r + 1) * 128],
                                        maskt[:],
                                    )
                            e2 = esb.tile([128, 2, QW], f16, tag="e2")
                            nc.scalar.activation(
                                e2[:, :, pre:], s2[:, :, pre:], Exp, scale=0.125
                            )
                            if pre:
                                nc.gpsimd.tensor_copy(
                                    e2[:, :, 0:pre],
                                    ones[:, 0:2 * pre].rearrange(
                                        "p (a b) -> p a b", a=2
                                    ),
                                )
                            for hl in range(2):
                                h = 2 * hp + hl
                                nc.tensor.matmul(
                                    ctx[hl][:],
                                    vA[:, kt_i, h, :],
                                    e2[:, hl, :],
                                    start=(kt_i == 0),
                                    stop=(kt_i == n_kt - 1 and last_on_loop),
                                )
                        if causal and cq < 3:
                            for hl in range(2):
                                h = 2 * hp + hl
                                nc.tensor.matmul(
                                    ctx[hl][0:DH + 1, :],
                                    vs_sb[0:1, cq, 128 * h:128 * h + DH + 1],
                                    ones[0:1, 0:QW],
                                    start=False, stop=True,
                                )
                        j = 2 * hp + cq // 2
                        for hl in range(2):
                            b = 32 * (2 * (cq % 2) + hl)
                            nc.vector.tensor_copy(
                                cN[64 * hl:64 * (hl + 1), hp, cq * QW:(cq + 1) * QW],
                                ctx[hl][0:DH, :],
                            )
                            nc.vector.tensor_copy(
                                zall[j][b:b + 1, :], ctx[hl][DH:DH + 1, :]
                            )
                        if cq % 2 == 1:
                            with nc.allow_low_precision(reason="fp16 recip"):
                                nc.vector.reciprocal(rzt[j][:], zall[j][:])
                            for cq2 in (cq - 1, cq):
                                for hl in range(2):
                                    b = 32 * (2 * (cq2 % 2) + hl)
                                    zb = zps.tile([DH, QW], f32, tag="zb")
                                    lo = 64 * hl
                                    nc.tensor.matmul(
                                        zb[:],
                                        ones[b:b + 1, 0:DH],
                                        rzt[j][b:b + 1, :],
                                        start=True, stop=True,
                                        tile_position=(b, 0),
                                    )
                                    nc.vector.tensor_mul(
                                        cT[lo:lo + DH, hp,
                                           cq2 * QW:(cq2 + 1) * QW],
                                        cN[lo:lo + DH, hp,
                                           cq2 * QW:(cq2 + 1) * QW],
                                        zb[:],
                                    )

            # ---------- Phase C: output projection (partial) ----------
            with (
                tc.tile_pool(name="ops", bufs=4, space="PSUM") as ops_p,
                tc.tile_pool(name="osb", bufs=3) as osb,
            ):
                for q_i in range(NKT):
                    ost = osb.tile([128, 2, QW], f32, tag="ost")
                    for no in range(2):
                        ps = ops_p.tile([128, QW], f32, tag="ops")
                        for ko in range(2):
                            nc.tensor.matmul(
                                ps[:],
                                cT[:, ko, q_i * 128:(q_i + 1) * 128],
                                wo_sb[:, ko, no * QW:(no + 1) * QW],
                                start=(ko == 0), stop=(ko == 1),
                            )
                        nc.vector.tensor_copy(ost[:, no, :], ps[:])
                    nc.sync.dma_start(
                        out_d[q_i * 128:(q_i + 1) * 128, :],
                        ost.rearrange("p a b -> p (a b)"),
                    )

    _split_sync_waits(nc)
    return nc


def _get_program(causal: bool):
    if causal not in _progs:
        _progs[causal] = _build_program(causal)
    return _progs[causal]


def _numpy_fallback(Q, K, V, pad_mask, attn_mask, Wq, bq, Wk, bk, Wv, bv, Wo, bo):
    NEG = np.float32(-1e-09)

    def split_heads(x):
        return x.reshape(B, S, NH, DH).transpose(0, 2, 1, 3)

    q = split_heads(Q @ Wq.T + bq)
    k = split_heads(K @ Wk.T + bk)
    v = split_heads(V @ Wv.T + bv)
    scores = np.einsum("bhqd,bhkd->bhqk", q, k) / np.sqrt(DH)
    mask = pad_mask[:, :, None] * pad_mask[:, None, :] * attn_mask
    scores = np.where(mask[:, None, :, :] != 0, scores, NEG)
    scores = scores - scores.max(axis=-1, keepdims=True)
    e = np.exp(scores)
    attn = e / e.sum(axis=-1, keepdims=True)
    ctx = np.einsum("bhqk,bhkd->bhqd", attn, v)
    ctx = ctx.transpose(0, 2, 1, 3).reshape(B, S, HID)
    return (ctx @ Wo.T + bo).astype(np.float32)


def kernel(Q, K, V, pad_mask, attn_mask, Wq, bq, Wk, bk, Wv, bv, Wo, bo):
    Q = np.asarray(Q, np.float32)
    K = np.asarray(K, np.float32)
    V = np.asarray(V, np.float32)
    pad_mask = np.asarray(pad_mask, np.float32)
    attn_mask = np.asarray(attn_mask, np.float32)
    Wq = np.asarray(Wq, np.float32)
    bq = np.asarray(bq, np.float32)
    Wk = np.asarray(Wk, np.float32)
    bk = np.asarray(bk, np.float32)
    Wv = np.asarray(Wv, np.float32)
    bv = np.asarray(bv, np.float32)
    Wo = np.asarray(Wo, np.float32)
    bo = np.asarray(bo, np.float32)

    tril = np.tril(np.ones((S, S), np.float32))
    if not np.all(pad_mask == 1.0):
        return _numpy_fallback(Q, K, V, pad_mask, attn_mask,
                               Wq, bq, Wk, bk, Wv, bv, Wo, bo)
    if np.array_equal(attn_mask, tril):
        causal = True
    elif np.all(attn_mask != 0.0):
        causal = False
    else:
        return _numpy_fallback(Q, K, V, pad_mask, attn_mask,
                               Wq, bq, Wk, bk, Wv, bv, Wo, bo)

    from concourse.bass_utils import run_bass_kernel_spmd

    nc = _get_program(causal)

    ones = np.ones((128, 2 * QW), np.float16)
    maskt = np.triu(np.ones((128, 128), np.float32))  # [key, query]: key<=query
    WqT = np.ascontiguousarray(Wq.T.astype(np.float16))
    WkT = np.ascontiguousarray(Wk.T.astype(np.float16))
    WvT = np.ascontiguousarray(Wv.T.astype(np.float16))
    WoT = np.ascontiguousarray(Wo.T.astype(np.float16))
    QT = [np.ascontiguousarray(Q[b].T.astype(np.float16)) for b in range(B)]
    KT = [np.ascontiguousarray(K[b].T.astype(np.float16)) for b in range(B)]
    VT = [np.ascontiguousarray(V[b].T.astype(np.float16)) for b in range(B)]

    in_maps = []
    for c in range(NCORES):
        b, g = divmod(c, GROUPS)
        hs = slice(g * HDS, (g + 1) * HDS)
        in_maps.append({
            "qt": QT[b], "kt": KT[b], "vt": VT[b],
            "wq": np.ascontiguousarray(WqT[:, hs]),
            "wk": np.ascontiguousarray(WkT[:, hs]),
            "wv": np.ascontiguousarray(WvT[:, hs]),
            "wo": np.ascontiguousarray(WoT[hs, :]),
            "bq": np.ascontiguousarray(bq[hs].reshape(2, 128).T),
            "bk": np.ascontiguousarray(bk[hs].reshape(2, 128).T),
            "bv": np.ascontiguousarray(bv[hs].reshape(1, HDS).astype(np.float16)),
            "on": ones, "mk": maskt,
        })

    global _trace_in_maps
    _trace_in_maps = in_maps

    res = run_bass_kernel_spmd(nc, in_maps, core_ids=list(range(NCORES)))
    out = np.empty((B, S, HID), np.float32)
    for b in range(B):
        acc = res.results[GROUPS * b]["out"].copy()
        for g in range(1, GROUPS):
            acc += res.results[GROUPS * b + g]["out"]
        out[b] = acc + bo
    return out


# revision 16
# speedup vs baseline: 1.4302x; 1.0839x over previous
"""Multi-head attention (B=2, S=2048, HID=1024, NH=16, DH=64) on 8 trn2 cores.

Sharding: tensor-parallel over (batch, head-group): core c handles batch c//4
and heads 4*(c%4)..4*(c%4)+3 (256 of the 1024 hidden dims). Each core computes
q/k/v projections for its heads, attention, and a partial output projection;
the host sums the 4 partials per batch and adds the output bias.

Layout strategy (matmul operands are fp16 — full PE clock, fast weight load,
fp32 PSUM accumulation; fp32->fp16 input rounding costs ~5e-4 relative error):
  - Host pre-transposes Q/K/V ([HID, S] per batch) and weights so no on-device
    transposes are needed.
  - qT, kT are kept head-major [dh, S]; scores are computed transposed
    (sT[key, query] = kT.T @ qT) with two heads packed into the 128-wide PE
    contraction via row-group tile_position.
  - exp(sT) tiles feed ctxT = v_aug.T @ expT where v_aug has a ones column, so
    the softmax denominator Z accumulates in PSUM row 64 for free.
  - The reference's mask quirk (masked scores set to -1e-9, NOT -inf) makes a
    masked entry contribute exp(-1e-9) == 1.0f; softmax max-subtraction is
    skipped (scores are O(1), exp is safe) so masked entries are exactly 1.0.
    Future-key blocks are therefore never computed: their contribution is a
    rank-1 update (suffix-sums of v_aug) added straight into the ctx PSUM.
"""

import numpy as np

B, S, HID, NH, DH = 2, 2048, 1024, 16, 64
NCORES = 8
GROUPS = 4            # head groups (cores per batch)
HPC = NH // GROUPS    # 4 heads per core
HDS = HPC * DH        # 256 hidden dims per core
QW = 512              # query-chunk width (one fp32 PSUM bank)
NCQ = S // QW         # 4 query chunks
NKT = S // 128        # 16 key tiles

_progs = {}


def _split_sync_waits(nc, max_waits: int = 1) -> int:
    """neuronxcc walrus codegen rejects instructions with more than one sync
    wait ("Too many sync wait commands"). Move excess waits onto preceding
    same-engine NoOps."""
    import concourse.mybir as mybir

    n_split = 0
    for fn in nc.m.functions:
        for bb in fn.blocks:
            out = []
            for ins in bb.instructions:
                si = ins.sync_info
                if si is not None and si.on_wait and len(si.on_wait) > max_waits:
                    waits = list(si.on_wait)
                    extra, keep = waits[:-max_waits], waits[-max_waits:]
                    for i in range(0, len(extra), max_waits):
                        chunk = extra[i:i + max_waits]
                        nop = mybir.InstNoOp(
                            name=nc.get_next_instruction_name(),
                            engine=ins.engine,
                            ins=[],
                            outs=[],
                            sync_info=mybir.SyncInfo(on_wait=chunk, on_update=[]),
                            bass_nofuse=True,
                            text_hint="split_sync_waits",
                        )
                        out.append(nop)
                        n_split += 1
                    si.on_wait = keep
                out.append(ins)
            bb.instructions[:] = out
    return n_split


def _build_program(causal: bool):
    import concourse.bass as bass
    import concourse.tile as tile
    from concourse import mybir

    f32 = mybir.dt.float32
    f16 = mybir.dt.float16
    Ident = mybir.ActivationFunctionType.Identity
    Copy = mybir.ActivationFunctionType.Copy
    Exp = mybir.ActivationFunctionType.Exp

    nc = bass.Bass()
    qt_d = nc.dram_tensor("qt", [HID, S], f16, kind="ExternalInput")
    kt_d = nc.dram_tensor("kt", [HID, S], f16, kind="ExternalInput")
    vt_d = nc.dram_tensor("vt", [HID, S], f16, kind="ExternalInput")
    wq_d = nc.dram_tensor("wq", [HID, HDS], f16, kind="ExternalInput")
    wk_d = nc.dram_tensor("wk", [HID, HDS], f16, kind="ExternalInput")
    wv_d = nc.dram_tensor("wv", [HID, HDS], f16, kind="ExternalInput")
    wo_d = nc.dram_tensor("wo", [HDS, HID], f16, kind="ExternalInput")
    bq_d = nc.dram_tensor("bq", [128, 2], f32, kind="ExternalInput")
    bk_d = nc.dram_tensor("bk", [128, 2], f32, kind="ExternalInput")
    bv_d = nc.dram_tensor("bv", [1, HDS], f16, kind="ExternalInput")
    on_d = nc.dram_tensor("on", [128, 2 * QW], f16, kind="ExternalInput")
    mk_d = nc.dram_tensor("mk", [128, 128], f32, kind="ExternalInput")
    out_d = nc.dram_tensor("out", [S, HID], f32, kind="ExternalOutput")

    qt_r = qt_d.rearrange("(ko p) s -> p ko s", p=128)
    kt_r = kt_d.rearrange("(ko p) s -> p ko s", p=128)
    vt_r = vt_d.rearrange("(ko p) s -> p ko s", p=128)

    with tile.TileContext(nc) as tc:
        with tc.tile_pool(name="persist", bufs=1) as persist:
            qT = persist.tile([128, 2, S], f16)       # [dh(2 heads), m, s]
            kT = persist.tile([128, 2, S], f16)
            vA = persist.tile([128, NKT, HPC, 128], f16)  # v_aug (padded to M=128)
            cT = persist.tile([128, 2, S], f16)       # ctxT (divided by Z)
            cN = persist.tile([128, 2, S], f16)       # ctxT numerator
            ones = persist.tile([128, 2 * QW], f16)
            maskt = persist.tile([128, 128], f32)
            bq_sb = persist.tile([128, 2], f32)
            bk_sb = persist.tile([128, 2], f32)
            bv_sb = persist.tile([1, HDS], f16)
            vs_sb = persist.tile([1, 3, HPC * 128], f16)
            wo_sb = persist.tile([128, 2, HID], f16)

            nc.sync.dma_start(ones[:], on_d[:])
            nc.sync.dma_start(maskt[:], mk_d[:])
            nc.sync.dma_start(bq_sb[:], bq_d[:])
            nc.sync.dma_start(bk_sb[:], bk_d[:])
            nc.sync.dma_start(bv_sb[:], bv_d[:])
            nc.sync.dma_start(wo_sb[:], wo_d.rearrange("(ko p) o -> p ko o", p=128))

            # ---------- Phase A: projections ----------
            with (
                tc.tile_pool(name="wpool", bufs=1) as wpool,
                tc.tile_pool(name="app", bufs=4, space="PSUM") as app,
                tc.tile_pool(name="vpp", bufs=2, space="PSUM") as vpp,
                tc.tile_pool(name="arhs", bufs=2) as arhs,
                tc.tile_pool(name="vsl", bufs=3) as vsl,
            ):
                wq_sb = wpool.tile([128, 8, HDS], f16, tag="wq")
                wk_sb = wpool.tile([128, 8, HDS], f16, tag="wk")
                wv_sb = wpool.tile([128, 8, HDS], f16, tag="wv")
                nc.sync.dma_start(wq_sb[:], wq_d.rearrange("(ko p) m -> p ko m", p=128))
                nc.sync.dma_start(wk_sb[:], wk_d.rearrange("(ko p) m -> p ko m", p=128))
                nc.sync.dma_start(wv_sb[:], wv_d.rearrange("(ko p) m -> p ko m", p=128))

                # zero pad columns + ones column (written once, before evacs)
                nc.vector.memset(vA[:, :, :, DH + 1:128], 0.0)
                nc.scalar.activation(
                    vA[:, :, :, DH:DH + 1],
                    ones[:, 0:NKT * HPC].rearrange("p (a b o) -> p a b o", a=NKT, b=HPC),
                    Copy,
                )
                for st in range(NKT):
                    vslab = vsl.tile([128, 8, 128], f16, tag="vslab")
                    nc.sync.dma_start(vslab[:], vt_r[:, :, st * 128:(st + 1) * 128])
                    ps = vpp.tile([128, HDS], f32, tag="vps")
                    for ko in range(8):
                        nc.tensor.matmul(
                            ps[:], vslab[:, ko, :], wv_sb[:, ko, :],
                            start=(ko == 0), stop=False,
                        )
                    nc.tensor.matmul(
                        ps[:], ones[0:1, 0:128], bv_sb[0:1, :],
                        start=False, stop=True,
                    )
                    nc.scalar.activation(
                        vA[:, st, :, 0:DH],
                        ps.rearrange("p (h d) -> p h d", h=HPC),
                        Ident,
                    )

                if causal:
                    # suffix sums of v_aug column-totals: vs_sb[0, c-1, :] =
                    # sum_{st >= 4c} colsum(v_aug[st])  (includes key counts)
                    for c in (1, 2, 3):
                        vps = vpp.tile([1, HPC * 128], f32, tag="vsps")
                        for st in range(4 * c, NKT):
                            nc.tensor.matmul(
                                vps[:],
                                ones[:, 0:1],
                                vA[:, st, :, :].rearrange("p a b -> p (a b)"),
                                start=(st == 4 * c),
                                stop=(st == NKT - 1),
                            )
                        nc.vector.tensor_copy(vs_sb[0:1, c - 1, :], vps[:])

                # q/k projections, per sequence chunk: attention on chunk 0
                # only needs its own q/k columns, so it overlaps chunks 1-3.
                for ns in range(4):
                    for src_r, w_sb, b_sb, dstT in (
                        (qt_r, wq_sb, bq_sb, qT),
                        (kt_r, wk_sb, bk_sb, kT),
                    ):
                        rh = arhs.tile([128, 8, QW], f16, tag="projrhs")
                        nc.sync.dma_start(rh[:], src_r[:, :, ns * QW:(ns + 1) * QW])
                        for m in range(2):
                            ps = app.tile([128, QW], f32, tag="projps")
                            for ko in range(8):
                                nc.tensor.matmul(
                                    ps[:],
                                    w_sb[:, ko, m * 128:(m + 1) * 128],
                                    rh[:, ko, :],
                                    start=(ko == 0),
                                    stop=(ko == 7),
                                )
                            nc.scalar.activation(
                                dstT[:, m, ns * QW:(ns + 1) * QW], ps[:],
                                Ident, bias=b_sb[:, m:m + 1],
                            )

            # ---------- Phase B: attention + output projection, per chunk ----
            # Z rows for chunk cq live at 32-aligned partitions of zall[cq] so
            # one batched DVE reciprocal + K=1 PE broadcasts recover 1/Z.
            zall = [persist.tile([97, QW], f32, name=f"zall{j}") for j in range(NCQ)]
            rzt = [persist.tile([97, QW], f16, name=f"rzt{j}") for j in range(NCQ)]
            with (
                tc.tile_pool(name="sps", bufs=2, space="PSUM") as sps,
                tc.tile_pool(name="cps", bufs=2, space="PSUM") as cps,
                tc.tile_pool(name="zps", bufs=1, space="PSUM") as zps,
                tc.tile_pool(name="ops", bufs=1, space="PSUM") as ops_p,
                tc.tile_pool(name="esb", bufs=4) as esb,
                tc.tile_pool(name="osb", bufs=3) as osb,
            ):
                for cq in range(NCQ):
                    n_kt = 4 * (cq + 1) if causal else NKT
                    for hp in range(2):
                        ctx = [
                            cps.tile([128, QW], f32, tag="ctx", name=f"ctx{i}")
                            for i in range(2)
                        ]
                        for kt_i in range(n_kt):
                            r = kt_i - 4 * cq
                            pre = r * 128 if (causal and r >= 1) else 0
                            s2 = sps.tile([128, 2, QW], f32, tag="s2")
                            for hl in range(2):
                                lo = 64 * hl
                                nc.tensor.matmul(
                                    s2[:, hl, pre:],
                                    kT[lo:lo + 64, hp, kt_i * 128:(kt_i + 1) * 128],
                                    qT[lo:lo + 64, hp, cq * QW + pre:(cq + 1) * QW],
                                    start=True, stop=True,
                                    tile_position=(lo, 0),
                                )
                            if causal and r >= 0:
                                for hl in range(2):
                                    nc.vector.tensor_mul(
                                        s2[:, hl, r * 128:(r + 1) * 128],
                                        s2[:, hl, r * 128:(r + 1) * 128],
                                        maskt[:],
                                    )
                            e2 = esb.tile([128, 2, QW], f16, tag="e2")
                            nc.scalar.activation(
                                e2[:, :, pre:], s2[:, :, pre:], Exp, scale=0.125
                            )
                            if pre:
                                nc.gpsimd.tensor_copy(
                                    e2[:, :, 0:pre],
                                    ones[:, 0:2 * pre].rearrange(
                                        "p (a b) -> p a b", a=2
                                    ),
                                )
                            for hl in range(2):
                                h = 2 * hp + hl
                                nc.tensor.matmul(
                                    ctx[hl][:],
                                    vA[:, kt_i, h, :],
                                    e2[:, hl, :],
                                    start=(kt_i == 0),
                                    stop=(kt_i == n_kt - 1 and
                                          ((not causal) or cq == 3)),
                                )
                        if causal and cq < 3:
                            for hl in range(2):
                                h = 2 * hp + hl
                                nc.tensor.matmul(
                                    ctx[hl][0:DH + 1, :],
                                    vs_sb[0:1, cq, 128 * h:128 * h + DH + 1],
                                    ones[0:1, 0:QW],
                                    start=False, stop=True,
                                )
                        for hl in range(2):
                            b = 32 * (2 * hp + hl)
                            nc.vector.tensor_copy(
                                cN[64 * hl:64 * (hl + 1), hp, cq * QW:(cq + 1) * QW],
                                ctx[hl][0:DH, :],
                            )
                            nc.vector.tensor_copy(
                                zall[cq][b:b + 1, :], ctx[hl][DH:DH + 1, :]
                            )
                    with nc.allow_low_precision(reason="fp16 recip"):
                        nc.vector.reciprocal(rzt[cq][:], zall[cq][:])
                    for hp in range(2):
                        for hl in range(2):
                            b = 32 * (2 * hp + hl)
                            lo = 64 * hl
                            zb = zps.tile([DH, QW], f32, tag="zb")
                            nc.tensor.matmul(
                                zb[:],
                                ones[b:b + 1, 0:DH],
                                rzt[cq][b:b + 1, :],
                                start=True, stop=True,
                                tile_position=(b, 0),
                            )
                            nc.vector.tensor_mul(
                                cT[lo:lo + DH, hp, cq * QW:(cq + 1) * QW],
                                cN[lo:lo + DH, hp, cq * QW:(cq + 1) * QW],
                                zb[:],
                            )
                    # output projection for this chunk's q-tiles
                    for qi in range(4):
                        q_i = 4 * cq + qi
                        ost = osb.tile([128, 2, QW], f32, tag="ost")
                        for no in range(2):
                            ps = ops_p.tile([128, QW], f32, tag="ops")
                            for ko in range(2):
                                nc.tensor.matmul(
                                    ps[:],
                                    cT[:, ko, q_i * 128:(q_i + 1) * 128],
                                    wo_sb[:, ko, no * QW:(no + 1) * QW],
                                    start=(ko == 0), stop=(ko == 1),
                                )
                            nc.vector.tensor_copy(ost[:, no, :], ps[:])
                        nc.sync.dma_start(
                            out_d[q_i * 128:(q_i + 1) * 128, :],
                            ost.rearrange("p a b -> p (a b)"),
                        )

    _split_sync_waits(nc)
    return nc


def _get_program(causal: bool):
    if causal not in _progs:
        _progs[causal] = _build_program(causal)
    return _progs[causal]


def _numpy_fallback(Q, K, V, pad_mask, attn_mask, Wq, bq, Wk, bk, Wv, bv, Wo, bo):
    NEG = np.float32(-1e-09)

    def split_heads(x):
        return x.reshape(B, S, NH, DH).transpose(0, 2, 1, 3)

    q = split_heads(Q @ Wq.T + bq)
    k = split_heads(K @ Wk.T + bk)
    v = split_heads(V @ Wv.T + bv)
    scores = np.einsum("bhqd,bhkd->bhqk", q, k) / np.sqrt(DH)
    mask = pad_mask[:, :, None] * pad_mask[:, None, :] * attn_mask
    scores = np.where(mask[:, None, :, :] != 0, scores, NEG)
    scores = scores - scores.max(axis=-1, keepdims=True)
    e = np.exp(scores)
    attn = e / e.sum(axis=-1, keepdims=True)
    ctx = np.einsum("bhqk,bhkd->bhqd", attn, v)
    ctx = ctx.transpose(0, 2, 1, 3).reshape(B, S, HID)
    return (ctx @ Wo.T + bo).astype(np.float32)


def kernel(Q, K, V, pad_mask, attn_mask, Wq, bq, Wk, bk, Wv, bv, Wo, bo):
    Q = np.asarray(Q, np.float32)
    K = np.asarray(K, np.float32)
    V = np.asarray(V, np.float32)
    pad_mask = np.asarray(pad_mask, np.float32)
    attn_mask = np.asarray(attn_mask, np.float32)
    Wq = np.asarray(Wq, np.float32)
    bq = np.asarray(bq, np.float32)
    Wk = np.asarray(Wk, np.float32)
    bk = np.asarray(bk, np.float32)
    Wv = np.asarray(Wv, np.float32)
    bv = np.asarray(bv, np.float32)
    Wo = np.asarray(Wo, np.float32)
    bo = np.asarray(bo, np.float32)

    tril = np.tril(np.ones((S, S), np.float32))
    if not np.all(pad_mask == 1.0):
        return _numpy_fallback(Q, K, V, pad_mask, attn_mask,
                               Wq, bq, Wk, bk, Wv, bv, Wo, bo)
    if np.array_equal(attn_mask, tril):
        causal = True
    elif np.all(attn_mask != 0.0):
        causal = False
    else:
        return _numpy_fallback(Q, K, V, pad_mask, attn_mask,
                               Wq, bq, Wk, bk, Wv, bv, Wo, bo)

    from concourse.bass_utils import run_bass_kernel_spmd

    nc = _get_program(causal)

    ones = np.ones((128, 2 * QW), np.float16)
    maskt = np.triu(np.ones((128, 128), np.float32))  # [key, query]: key<=query
    WqT = np.ascontiguousarray(Wq.T.astype(np.float16))
    WkT = np.ascontiguousarray(Wk.T.astype(np.float16))
    WvT = np.ascontiguousarray(Wv.T.astype(np.float16))
    WoT = np.ascontiguousarray(Wo.T.astype(np.float16))
    QT = [np.ascontiguousarray(Q[b].T.astype(np.float16)) for b in range(B)]
    KT = [np.ascontiguousarray(K[b].T.astype(np.float16)) for b in range(B)]
    VT = [np.ascontiguousarray(V[b].T.astype(np.float16)) for b in range(B)]

    in_maps = []
    for c in range(NCORES):
        b, g = divmod(c, GROUPS)
        hs = slice(g * HDS, (g + 1) * HDS)
        in_maps.append({
            "qt": QT[b], "kt": KT[b], "vt": VT[b],
            "wq": np.ascontiguousarray(WqT[:, hs]),
            "wk": np.ascontiguousarray(WkT[:, hs]),
            "wv": np.ascontiguousarray(WvT[:, hs]),
            "wo": np.ascontiguousarray(WoT[hs, :]),
            "bq": np.ascontiguousarray(bq[hs].reshape(2, 128).T),
            "bk": np.ascontiguousarray(bk[hs].reshape(2, 128).T),
            "bv": np.ascontiguousarray(bv[hs].reshape(1, HDS).astype(np.float16)),
            "on": ones, "mk": maskt,
        })

    global _trace_in_maps
    _trace_in_maps = in_maps

    res = run_bass_kernel_spmd(nc, in_maps, core_ids=list(range(NCORES)))
    out = np.empty((B, S, HID), np.float32)
    for b in range(B):
        acc = res.results[GROUPS * b]["out"].copy()
        for g in range(1, GROUPS):
            acc += res.results[GROUPS * b + g]["out"]
        out[b] = acc + bo
    return out
